# revision 22
# baseline (speedup 1.0000x reference)
"""Distributed kNN classifier for Trainium2 (8 NeuronCores).

Strategy
--------
reference(...) computes sim = feature @ feature_bank  [B, N], takes top-k
(k=200) per query, exp(sim/0.1) weights, scatter-adds into per-class scores
and returns the descending stable argsort of those scores.

The heavy part is the [1024, 1024] @ [1024, 100000] matmul plus top-k.
feature_bank is sharded along N across the 8 cores (12500 cols each).

Device (default, fp8): each core computes its sim shard with an fp8e4m3
DoubleRow matmul (fp32 PSUM accumulation, 2 MACs/cell/cycle) and writes
uint8 `clamp(round(sim - 64), 0, 255)` — candidate mask and coarse value in
one byte.  Sims are ~N(0, 32^2); every query's true 200th-largest sim is
>= ~84, and the fp8 matmul error is bounded by E_FP8, so the candidates
with stored value >= 10 (sim >~ 74) are a guaranteed superset of the true
top-k.  The host then (a) keeps, per query, only candidates
within 2*E of the device-value 200th-largest (a confidence window that
provably contains the true top-k), (b) recomputes exact fp32 similarities
for those ~0.4% of pairs, (c) selects the exact top-k with jax.lax.top_k
tie semantics and replicates the reference's exp/scatter/argsort in numpy.
If any query yields fewer than k candidates, the host falls back to an
exact full-row recompute for it, so correctness never depends on the
threshold.

Device-kernel layout: inputs are host-packed so d-row r lives on
partition r%128, plane r//128.  featT is two [128, 8, 512] query-halves
(one DMA each on the sync HWDGE ring); the bank shard is loaded in
ramp-up chunks (256/512/1024 then 2048-wide, one DMA per chunk on the
scalar HWDGE ring) so the first matmul only gates on ~0.75 MiB.  The
first two chunks are duplicated in a contiguous-per-partition side
input (bank8head) for wide DMA lines; later chunks read the
plane-strided bank8 so their packets stay small and never starve the
featT transfers.  The gate-critical loads (featT + chunk0) are issued
as raw bass before the TileContext entry barrier (manual semaphores,
waits attached post-scheduling), and the bulk prefetch is need-paced
(chunk1/2 DMAs held until the gate lands, bankp bufs=2) because the two
NeuronCores of an HBM stack share ~716 GB/s and an eager 13 MB burst on
one core starves its neighbour's gate.  During the DMA prologue,
N_WARM_MM dummy matmuls on raw uninitialized SBUF keep the PE busy
through the HAM activity window (cold K=4/8 -> warm K=8/8), so the real
MM stream runs at the warm 216 ns/MM roofline pace from its first
instruction.  The final subtile uses separate per-query-half PSUM/val
tiles and stores on both HWDGE rings so its qh0 postprocessing drains
while the qh1 matmuls still run.  Measured: ~169.5 us of back-to-back
matmuls at the fp8 DoubleRow roofline + ~12 us start + ~12 us fixed
framework preamble/teardown => ~193-195 us/core (baseline ~202 us).

A bf16 variant (KNN_IMPL=bf16) with a uint8 `sim > T0` mask output is kept
as a fallback, as is the unpacked fp8 layout (KNN_IMPL=fp8v0).
"""

import os
import sys
import time
import numpy as np
import ml_dtypes


def _tlog(msg, _t=[None]):
    if os.environ.get("KNN_TIMING"):
        now = time.time()
        dt = 0.0 if _t[0] is None else now - _t[0]
        _t[0] = now
        print(f"[knn +{dt:6.2f}s] {msg}", file=sys.stderr, flush=True)


import concourse.bass as bass
import concourse.bacc as bacc
import concourse.mybir as mybir
from concourse import tile
from concourse.bass_utils import run_bass_kernel_spmd

# Problem geometry (hardcoded per spec).
B = 1024          # queries
D = 1024          # feature dim
N_TOTAL = 100000  # bank size
N_CORES = 8
N_SHARD = N_TOTAL // N_CORES  # 12500

P = 128           # partitions
KCH = D // P      # 8 contraction planes (one per 128 d-rows)
KK = D // (2 * P)  # 4 DoubleRow contraction chunks (fp8)
QW = 512          # rhs free width per matmul (one PSUM bank of fp32)
CH = 512          # bank columns loaded per DMA chunk (bf16 path)

T0 = 80.0         # bf16 mask threshold (true 200th-largest sim is >= ~84.2)
VAL_OFF = 64.0    # u8 value-output offset: stored = clamp(sim - 64, 0, 255)
T0_FP8_U8 = 10    # u8 threshold (sim >~ 74; fp8 |err| <= ~6.6, margin ~10)
E_FP8 = 8.5       # fp8 matmul + u8 quantization error bound for the window

# fp8 kernel chunking: ramp-up chunks so matmuls start as soon as ~0.75 MiB
# has landed and each later chunk arrives before compute reaches it
# (sum == N_SHARD).
CHUNKS_V2 = (256, 512, 1024, 2048, 2048, 2048, 2048, 2048, 468)
CH_ALLOC_V2 = 2048  # SBUF tile width for big chunks (plane stride, 16-aligned)
N_WARM_MM = 14      # dummy MMs to warm the PE HAM gate during the DMA prologue

# v3: finer gate granularity.  c0/c1 are single 128-col subtiles so the first
# matmul gates on only ~0.4 MiB; featT halves stream in per-plane-pair so each
# contraction step's data can land just-in-time (sum == N_SHARD).
CHUNKS_V3 = (128, 128, 256, 512, 1024, 2048, 2048, 2048, 2048, 2048, 212)
N_WARM_V3 = (7, 1)  # (N=512 warmup MMs, N=128 warmup MMs): cover ~6.1->9.2us

# v4: query-pass split with the whole bank shard resident in SBUF.
# The 12.5MB shard fits at ~100KB/partition, so each column is loaded once and
# consumed twice (pass qh0, then pass qh1).  The matmul stream then gates on
# only fq0 (512KiB, split across both rings) + the first 128-col chunk, and
# fq1 lands any time during the ~85us first pass.
N_RES = 12512       # resident bank columns alloc (12500 padded to %16 == 0)
N_WARM_V4 = (12, 2)  # warmups cover ~6.3us -> gate land ~11.4us, no PE idle gap

# v5: per-chunk dual-pass.  v4's single-qh passes consume bank columns at 2x
# the v2 rate (6.75 ns/col) -- more than one HWDGE ring can deliver -- which
# starved the stream (28us stall).  v5 instead consumes each chunk for BOTH
# query halves back-to-back (13.5 ns/col, the proven v2 rate), except for a
# q0-only ramp over chunks 0-3 (640 cols) while fq1 is still in flight; their
# q1 sweeps run at the very end from resident SBUF (DMA-free tail).
CHUNKS_V5 = (128, 128, 128, 256, 512, 1024, 2048, 2048, 2048, 2048, 2048, 84)
N_RAMP_V5 = 5        # chunks 0..4 run q0-only first, q1 deferred to the tail
N_WARM_V5 = (10, 1)  # warmups cover ~6.1us -> gate land ~10.5us

KNN_T = 0.1

LAST_EXEC_TIME_NS = None
LAST_DEV_VALS = None  # [N, B] device sims (fp8 path), for diagnostics


def _build_program_bf16(n_shard: int = N_SHARD):
    """bf16 matmul; uint8 mask output."""
    nc = bacc.Bacc("TRN2", target_bir_lowering=False, debug=False)

    featT = nc.dram_tensor("featT", [D, B], mybir.dt.bfloat16, kind="ExternalInput")
    bank = nc.dram_tensor("bank", [D, n_shard], mybir.dt.bfloat16, kind="ExternalInput")
    mask = nc.dram_tensor("mask", [n_shard, B], mybir.dt.uint8, kind="ExternalOutput")

    with tile.TileContext(nc) as tc:
        with (
            tc.tile_pool(name="feat", bufs=1) as feat_pool,
            tc.tile_pool(name="bankp", bufs=4) as bank_pool,
            tc.tile_pool(name="maskp", bufs=6) as mask_pool,
            tc.tile_pool(name="psum", bufs=6, space=bass.MemorySpace.PSUM) as psum_pool,
        ):
            # All of feature^T stays resident: [128, 8, 1024] bf16 (16 KiB/part)
            featT_sb = feat_pool.tile([P, KCH, B], mybir.dt.bfloat16)
            for kc in range(KCH):
                nc.sync.dma_start(featT_sb[:, kc, :], featT[kc * P:(kc + 1) * P, :])

            nch = (n_shard + CH - 1) // CH
            for ci in range(nch):
                c0 = ci * CH
                cw = min(CH, n_shard - c0)
                bank_sb = bank_pool.tile([P, KCH, CH], mybir.dt.bfloat16)
                for kc in range(KCH):
                    nc.sync.dma_start(
                        bank_sb[:, kc, :cw], bank[kc * P:(kc + 1) * P, c0:c0 + cw]
                    )
                for si in range(0, cw, P):
                    sw = min(P, cw - si)
                    mask_t = mask_pool.tile([P, B], mybir.dt.uint8)
                    for qh in range(B // QW):
                        ps = psum_pool.tile([P, QW], mybir.dt.float32)
                        for kc in range(KCH):
                            nc.tensor.matmul(
                                ps[:sw, :],
                                bank_sb[:, kc, si:si + sw],
                                featT_sb[:, kc, qh * QW:(qh + 1) * QW],
                                start=(kc == 0),
                                stop=(kc == KCH - 1),
                            )
                        nc.vector.tensor_scalar(
                            out=mask_t[:sw, qh * QW:(qh + 1) * QW],
                            in0=ps[:sw, :],
                            scalar1=T0,
                            scalar2=None,
                            op0=mybir.AluOpType.is_gt,
                        )
                    nc.sync.dma_start(mask[c0 + si:c0 + si + sw, :], mask_t[:sw, :])

    nc.compile()
    return nc


def _build_program_fp8(n_shard: int = N_SHARD):
    """fp8e4m3 DoubleRow matmul, host-packed layout (v2).

    Inputs (packed on host; d-row r -> partition r % 128, plane r // 128):
      featT8 [128, 2, 8, 512]: featT8[p, qh, j, c] = feature[512*qh + c, 128*j + p]
      bank8  [128, 8 * n_shard]: chunk-major — for chunk (c0, cw), partition p
        holds the chunk's 8 planes contiguously ([8, cw] row-major), so each
        chunk is one contiguous-per-partition DMA with >= 2 KiB lines.
    Output: vals [n_shard, B] u8 = clamp(round(sim - VAL_OFF), 0, 255).
    """
    assert n_shard == sum(CHUNKS_V2)
    nc = bacc.Bacc("TRN2", target_bir_lowering=False, debug=False)

    featT8 = nc.dram_tensor(
        "featT8", [P, 2, KCH, QW], mybir.dt.float8e4, kind="ExternalInput"
    )
    bank8 = nc.dram_tensor(
        "bank8", [P, KCH, n_shard], mybir.dt.float8e4, kind="ExternalInput"
    )
    head_cols = CHUNKS_V2[0] + CHUNKS_V2[1]
    bank8head = nc.dram_tensor(
        "bank8head", [P, KCH * head_cols], mybir.dt.float8e4,
        kind="ExternalInput",
    )
    vals = nc.dram_tensor("vals", [n_shard, B], mybir.dt.uint8, kind="ExternalOutput")

    DR = mybir.MatmulPerfMode.DoubleRow

    # PE warmup operands outside the Tile world: raw (uninitialized) SBUF as
    # garbage source, and a PSUM bank that is freed back to the stack before
    # the tile pools claim all 8 banks.  Safe because the in-order PE queue
    # runs every warmup MM before the first real MM, whose start=True clears
    # the bank's has_written state.
    wsrc = nc.alloc_sbuf_tensor("warm_src", [P, 2, QW], mybir.dt.float8e4)
    wsrc_ap = wsrc[:, :, :]

    # Gate-critical loads issued as raw bass BEFORE the TileContext entry
    # barrier, so their ring transfers start ~1.3 us earlier than any
    # tile-managed DMA could: featT halves on the sync ring, chunk0 on the
    # scalar ring.  Completion is signalled on manual semaphores; the waits
    # are attached to the first consuming matmuls AFTER the Tile scheduler
    # runs (it cannot model producers outside its block).
    cw0, cw1 = CHUNKS_V2[0], CHUNKS_V2[1]
    fq0_r = nc.alloc_sbuf_tensor("fq0_r", [P, KCH, QW], mybir.dt.float8e4)
    fq1_r = nc.alloc_sbuf_tensor("fq1_r", [P, KCH, QW], mybir.dt.float8e4)
    c0_r = nc.alloc_sbuf_tensor("c0_r", [P, KCH, cw0], mybir.dt.float8e4)
    s_fq0 = nc.alloc_semaphore("s_fq0")
    s_fq1 = nc.alloc_semaphore("s_fq1")
    s_c0 = nc.alloc_semaphore("s_c0")
    nc.sync.dma_start(fq0_r[:, :, :], featT8[:, 0, :, :]).then_inc(s_fq0, 16)
    nc.sync.dma_start(fq1_r[:, :, :], featT8[:, 1, :, :]).then_inc(s_fq1, 16)
    nc.scalar.dma_start(
        c0_r[:, :, :], bank8head[:, 0:KCH * cw0]
    ).then_inc(s_c0, 16)
    # Placeholder waits (sem >= 0, trivially satisfied so the Tile
    # scheduler's simulator never blocks on them); upgraded to >= 16 after
    # scheduling.  In-order engine queues make one wait per sem sufficient.
    gate_waits = []  # (BassInstruction placeholder, sem)
    defer_dmas = []  # chunk-prefetch DMAs held until the gate completes

    # Warmup MMs BEFORE the TileContext: they start right after the Tensor
    # engine's framework preamble (~0.7 us earlier than in-context), so the
    # HAM window fires sooner.  The context entry barrier then waits for the
    # warmup drain (~12.3 us) — harmless, since everything it gates (chunk
    # prefetch issues, DVE, stores) is needed only after the data gate.
    with nc.psum_tensor([P, QW], mybir.dt.float32) as wps:
        wps_ap = wps[:, :]
        for _ in range(N_WARM_MM):
            nc.tensor.matmul(
                wps_ap, wsrc_ap[:, :, 0:P], wsrc_ap,
                start=True, stop=True, perf_mode=DR,
            )

    with tile.TileContext(nc) as tc:
        gate_waits.append((nc.tensor.wait_ge(s_c0, 0), s_c0))
        gate_waits.append((nc.tensor.wait_ge(s_fq0, 0), s_fq0))
        with (
            tc.tile_pool(name="feat", bufs=1) as feat_pool,
            tc.tile_pool(name="b256", bufs=1) as b256_pool,
            tc.tile_pool(name="b512", bufs=1) as b512_pool,
            tc.tile_pool(name="b1024", bufs=1) as b1024_pool,
            tc.tile_pool(name="bankp", bufs=2) as bank_pool,
            tc.tile_pool(name="valp", bufs=6) as val_pool,
            tc.tile_pool(name="psum", bufs=4, space=bass.MemorySpace.PSUM) as psum_pool,
        ):
            # All loads go on the sync HWDGE ring in gate-priority order
            # (FIFO per ring): featT qh0, bank chunk0, featT qh1, chunks 1+.
            # Stores use the scalar ring so they never queue ahead of loads.
            # Distinct callsites: same-line pool.tile() calls share a tag and
            # would alias the same buffer (deadlock).
            small_pools = {256: b256_pool, 512: b512_pool, 1024: b1024_pool}
            bank_tiles = {}

            def new_ps():
                # Shared callsite -> one pool tag -> one rotating buffer set.
                ps = psum_pool.tile([P, B], mybir.dt.float32)
                return ps

            def new_val():
                val_t = val_pool.tile([P, B], mybir.dt.uint8)
                return val_t

            bank_tiles[0] = c0_r
            featq = [fq0_r, fq1_r]

            bank_c1 = small_pools[cw1].tile([P, KCH, cw1], mybir.dt.float8e4)
            bank_tiles[1] = bank_c1
            defer_dmas.append(nc.scalar.dma_start(
                bank_c1[:, :, :], bank8head[:, KCH * cw0:KCH * (cw0 + cw1)]
            ))

            n_subtiles = sum((cw + P - 1) // P for cw in CHUNKS_V2)
            sub_i = 0
            c0 = 0
            for ci, cw in enumerate(CHUNKS_V2):
                if ci <= 1:
                    bank_sb = bank_tiles[ci]
                else:
                    if ci < 3:
                        ca = cw
                        pool = small_pools[cw]
                    else:
                        ca = CH_ALLOC_V2
                        pool = bank_pool
                    bank_sb = pool.tile([P, KCH, ca], mybir.dt.float8e4)
                    # One DMA per chunk (plane-strided source); the HW
                    # splits it across all 16 SDMA engines.
                    dma = nc.scalar.dma_start(
                        bank_sb[:, :, :cw], bank8[:, :, c0:c0 + cw]
                    )
                    if ci == 2:
                        defer_dmas.append(dma)
                for si in range(0, cw, P):
                    sw = min(P, cw - si)
                    sub_i += 1
                    if sub_i < n_subtiles:
                        ps = new_ps()
                        val_t = new_val()
                        for qh in range(2):
                            if ci == 0 and si == 0 and qh == 1:
                                gate_waits.append(
                                    (nc.tensor.wait_ge(s_fq1, 0), s_fq1)
                                )
                            for kk in range(KK):
                                nc.tensor.matmul(
                                    ps[:sw, qh * QW:(qh + 1) * QW],
                                    bank_sb[:, 2 * kk:2 * kk + 2, si:si + sw],
                                    featq[qh][:, 2 * kk:2 * kk + 2, :],
                                    start=(kk == 0),
                                    stop=(kk == KK - 1),
                                    perf_mode=DR,
                                )
                        # (sim - 64) clamped at 0, cast to u8 — single DVE op.
                        nc.vector.tensor_scalar(
                            out=val_t[:sw, :],
                            in0=ps[:sw, :],
                            scalar1=-VAL_OFF,
                            scalar2=0.0,
                            op0=mybir.AluOpType.add,
                            op1=mybir.AluOpType.max,
                        )
                        nc.sync.dma_start(
                            vals[c0 + si:c0 + si + sw, :], val_t[:sw, :]
                        )
                    else:
                        # Final subtile: separate psum tiles per query half so
                        # the qh0 DVE+store drain while the qh1 MMs still run
                        # (a shared tile would add a whole-tile WAR edge).
                        ps_a = new_ps()
                        ps_b = new_ps()
                        val_a = new_val()
                        val_b = new_val()
                        for qh, ps_h, val_h in ((0, ps_a, val_a), (1, ps_b, val_b)):
                            for kk in range(KK):
                                nc.tensor.matmul(
                                    ps_h[:sw, 0:QW],
                                    bank_sb[:, 2 * kk:2 * kk + 2, si:si + sw],
                                    featq[qh][:, 2 * kk:2 * kk + 2, :],
                                    start=(kk == 0),
                                    stop=(kk == KK - 1),
                                    perf_mode=DR,
                                )
                            qs = slice(qh * QW, (qh + 1) * QW)
                            nc.vector.tensor_scalar(
                                out=val_h[:sw, 0:QW], in0=ps_h[:sw, 0:QW],
                                scalar1=-VAL_OFF, scalar2=0.0,
                                op0=mybir.AluOpType.add, op1=mybir.AluOpType.max,
                            )
                            eng = nc.scalar if qh == 0 else nc.sync
                            eng.dma_start(
                                vals[c0 + si:c0 + si + sw, qs], val_h[:sw, 0:QW]
                            )
                c0 += cw

    # Upgrade the placeholder waits now that the Tile scheduler has run (it
    # cannot model producers outside its block and would report a deadlock).
    for w, sem in gate_waits:
        w.wait_op(sem, 16, "sem-ge")
    # Hold the chunk1/chunk2 prefetches until the gate set has landed: the
    # two NeuronCores of an HBM stack share ~716 GB/s, so an eager prefetch
    # on one core starves its neighbour's gate transfers (seen as a +3 us
    # late start).  Later chunks queue behind chunk2 on the scalar ring.
    for dma in defer_dmas:
        dma.wait_op(s_c0, 16, "sem-ge")

    nc.compile()
    return nc


def _strip_init_overhead(nc):
    """Drop the const-tile memsets + init all-engine barrier that Bass.__init__
    emits into block 0.  Nothing in this kernel reads the const tiles, and the
    per-engine NEFF prologue plus the TileContext entry barrier provide all the
    cross-engine ordering the kernel needs.  Verified on HW with a minimal
    kernel (no hang, correct results); saves ~1us of start latency by letting
    the Scalar/Sync engines reach the gate DMA issues sooner."""
    blk = nc.main_func.blocks[0]
    keep = []
    for inst in blk.instructions:
        nm = str(getattr(inst, "name", ""))
        if (
            isinstance(inst, (mybir.InstMemset, mybir.InstDrain))
            or nm.startswith("barrier_")
        ):
            continue
        keep.append(inst)
    blk.instructions[:] = keep


def _build_program_fp8_v3(n_shard: int = N_SHARD):
    """fp8e4m3 DoubleRow matmul, v3 start/tail schedule.

    Inputs (packed on host; d-row r -> partition r % 128, plane r // 128):
      featT8  [128, 2, 8, 512]: featT8[p, qh, j, c] = feature[512*qh + c, 128*j + p]
      bank8cm [128, 8 * n_shard]: chunk-major -- for each chunk of CHUNKS_V3 at
        column offset o, width cw, the slab bank8cm[:, 8*o : 8*(o+cw)] holds the
        chunk's 8 planes contiguously per partition ([8, cw] row-major), so
        every chunk load is one contiguous-per-partition DMA.
    Output: vals [n_shard, B] u8 = clamp(round(sim - VAL_OFF), 0, 255).

    Differences vs v2 (see module docstring for the shared scheme):
      * Init memsets + barrier stripped (_strip_init_overhead): gate DMAs issue
        at ~6.2us instead of ~7.2us.
      * featT halves stream per-plane-pair (4 DMAs each, own sems), and the
        first real matmuls gate per contraction step, so the real MM stream
        starts as soon as ~0.4 MiB lands (~9.2us) and runs its first few MMs
        cold instead of burning that window on warmup garbage.
      * Ring split: scalar = c0, fq0 pieces, c2 + chunks c3+; sync = c1, fq1
        pieces + all stores.  Both rings carry ~equal gate bytes.
      * Final subtile's qh1 drains as two 256-col DVE+store pieces on both
        rings to shorten the post-stream drain.
    """
    assert n_shard == sum(CHUNKS_V3)
    nc = bacc.Bacc("TRN2", target_bir_lowering=False, debug=False)
    _strip_init_overhead(nc)

    featT8 = nc.dram_tensor(
        "featT8", [P, 2, KCH, QW], mybir.dt.float8e4, kind="ExternalInput"
    )
    bank8cm = nc.dram_tensor(
        "bank8cm", [P, KCH * n_shard], mybir.dt.float8e4, kind="ExternalInput"
    )
    vals = nc.dram_tensor("vals", [n_shard, B], mybir.dt.uint8, kind="ExternalOutput")

    DR = mybir.MatmulPerfMode.DoubleRow

    # PE warmup operands outside the Tile world (see v2 docstring).
    wsrc = nc.alloc_sbuf_tensor("warm_src", [P, 2, QW], mybir.dt.float8e4)
    wsrc_ap = wsrc[:, :, :]

    # Raw gate loads, issued before the TileContext entry barrier.  Order
    # within each engine queue is earliest-need-first; completion is signalled
    # on manual semaphores, with waits attached post-scheduling.
    cw0, cw1, cw2 = CHUNKS_V3[0], CHUNKS_V3[1], CHUNKS_V3[2]
    fq0_r = nc.alloc_sbuf_tensor("fq0_r", [P, KCH, QW], mybir.dt.float8e4)
    fq1_r = nc.alloc_sbuf_tensor("fq1_r", [P, KCH, QW], mybir.dt.float8e4)
    c0_r = nc.alloc_sbuf_tensor("c0_r", [P, KCH, cw0], mybir.dt.float8e4)
    c1_r = nc.alloc_sbuf_tensor("c1_r", [P, KCH, cw1], mybir.dt.float8e4)
    c2_r = nc.alloc_sbuf_tensor("c2_r", [P, KCH, cw2], mybir.dt.float8e4)
    s_f0 = [nc.alloc_semaphore(f"s_f0_{k}") for k in range(KK)]
    s_f1 = [nc.alloc_semaphore(f"s_f1_{k}") for k in range(KK)]
    s_c0 = nc.alloc_semaphore("s_c0")
    s_c1 = nc.alloc_semaphore("s_c1")
    s_c2 = nc.alloc_semaphore("s_c2")

    # scalar ring: c0 first (gates MM #1 with f0[0]), then fq0 plane-pairs,
    # then c2.  Chunk-major bank slabs are contiguous per partition.
    nc.scalar.dma_start(c0_r[:, :, :], bank8cm[:, 0:KCH * cw0]).then_inc(s_c0, 16)
    for kk in range(KK):
        nc.scalar.dma_start(
            fq0_r[:, 2 * kk:2 * kk + 2, :], featT8[:, 0, 2 * kk:2 * kk + 2, :]
        ).then_inc(s_f0[kk], 16)
    o1 = cw0
    o2 = cw0 + cw1
    nc.scalar.dma_start(
        c2_r[:, :, :], bank8cm[:, KCH * o2:KCH * (o2 + cw2)]
    ).then_inc(s_c2, 16)
    # sync ring: c1, then fq1 plane-pairs (needed from ~T0+0.9us onward).
    nc.sync.dma_start(c1_r[:, :, :], bank8cm[:, KCH * o1:KCH * (o1 + cw1)]).then_inc(
        s_c1, 16
    )
    for kk in range(KK):
        nc.sync.dma_start(
            fq1_r[:, 2 * kk:2 * kk + 2, :], featT8[:, 1, 2 * kk:2 * kk + 2, :]
        ).then_inc(s_f1[kk], 16)

    gate_waits = []  # (BassInstruction placeholder, sem) -> upgraded to >= 16

    # Warmup MMs keep the PE HAM activity window alive from ~6.1us until the
    # gate data lands (~9.2us); the first real MMs then run cold but do real
    # work while the HAM finishes warming.
    with nc.psum_tensor([P, QW], mybir.dt.float32) as wps:
        wps_ap = wps[:, :]
        for _ in range(N_WARM_V3[0]):
            nc.tensor.matmul(
                wps_ap, wsrc_ap[:, :, 0:P], wsrc_ap,
                start=True, stop=True, perf_mode=DR,
            )
        for _ in range(N_WARM_V3[1]):
            nc.tensor.matmul(
                wps_ap[:, 0:P], wsrc_ap[:, :, 0:P], wsrc_ap[:, :, 0:P],
                start=True, stop=True, perf_mode=DR,
            )

    with tile.TileContext(nc) as tc:
        with (
            tc.tile_pool(name="b512", bufs=2) as b512_pool,
            tc.tile_pool(name="b1024", bufs=1) as b1024_pool,
            tc.tile_pool(name="bankp", bufs=2) as bank_pool,
            tc.tile_pool(name="valp", bufs=6) as val_pool,
            tc.tile_pool(name="psum", bufs=4, space=bass.MemorySpace.PSUM) as psum_pool,
        ):
            raw_chunks = {0: c0_r, 1: c1_r, 2: c2_r}
            featq = [fq0_r, fq1_r]

            def new_ps():
                ps = psum_pool.tile([P, B], mybir.dt.float32)
                return ps

            def new_val():
                val_t = val_pool.tile([P, B], mybir.dt.uint8)
                return val_t

            n_subtiles = sum((cw + P - 1) // P for cw in CHUNKS_V3)
            sub_i = 0
            c0 = 0
            for ci, cw in enumerate(CHUNKS_V3):
                if ci in raw_chunks:
                    bank_sb = raw_chunks[ci]
                else:
                    if cw <= 512:
                        pool, ca = b512_pool, 512
                    elif cw <= 1024:
                        pool, ca = b1024_pool, 1024
                    else:
                        pool, ca = bank_pool, 2048
                    bank_sb = pool.tile([P, KCH, ca], mybir.dt.float8e4)
                    nc.scalar.dma_start(
                        bank_sb[:, :, :cw], bank8cm[:, KCH * c0:KCH * (c0 + cw)]
                    )
                for si in range(0, cw, P):
                    sw = min(P, cw - si)
                    sub_i += 1
                    if sub_i < n_subtiles:
                        ps = new_ps()
                        val_t = new_val()
                        for qh in range(2):
                            for kk in range(KK):
                                if ci == 0 and si == 0:
                                    if qh == 0 and kk == 0:
                                        gate_waits.append(
                                            (nc.tensor.wait_ge(s_c0, 0), s_c0)
                                        )
                                    sem = (s_f0, s_f1)[qh][kk]
                                    gate_waits.append(
                                        (nc.tensor.wait_ge(sem, 0), sem)
                                    )
                                elif ci in (1, 2) and si == 0 and qh == 0 and kk == 0:
                                    sem = s_c1 if ci == 1 else s_c2
                                    gate_waits.append(
                                        (nc.tensor.wait_ge(sem, 0), sem)
                                    )
                                nc.tensor.matmul(
                                    ps[:sw, qh * QW:(qh + 1) * QW],
                                    bank_sb[:, 2 * kk:2 * kk + 2, si:si + sw],
                                    featq[qh][:, 2 * kk:2 * kk + 2, :],
                                    start=(kk == 0),
                                    stop=(kk == KK - 1),
                                    perf_mode=DR,
                                )
                        nc.vector.tensor_scalar(
                            out=val_t[:sw, :],
                            in0=ps[:sw, :],
                            scalar1=-VAL_OFF,
                            scalar2=0.0,
                            op0=mybir.AluOpType.add,
                            op1=mybir.AluOpType.max,
                        )
                        nc.sync.dma_start(
                            vals[c0 + si:c0 + si + sw, :], val_t[:sw, :]
                        )
                    else:
                        # Final subtile: qh0 drains on the scalar ring while
                        # qh1's MMs run; qh1 then drains as two 256-col pieces
                        # on both rings so the post-stream chain is
                        # DVE[84,256] + a smaller store on each ring.
                        ps_a = new_ps()
                        ps_b = new_ps()
                        val_a = new_val()
                        val_b = new_val()
                        for qh, ps_h in ((0, ps_a), (1, ps_b)):
                            for kk in range(KK):
                                nc.tensor.matmul(
                                    ps_h[:sw, 0:QW],
                                    bank_sb[:, 2 * kk:2 * kk + 2, si:si + sw],
                                    featq[qh][:, 2 * kk:2 * kk + 2, :],
                                    start=(kk == 0),
                                    stop=(kk == KK - 1),
                                    perf_mode=DR,
                                )
                        nc.vector.tensor_scalar(
                            out=val_a[:sw, 0:QW], in0=ps_a[:sw, 0:QW],
                            scalar1=-VAL_OFF, scalar2=0.0,
                            op0=mybir.AluOpType.add, op1=mybir.AluOpType.max,
                        )
                        nc.scalar.dma_start(
                            vals[c0 + si:c0 + si + sw, 0:QW], val_a[:sw, 0:QW]
                        )
                        half = QW // 2
                        for pi, eng in ((0, nc.sync), (1, nc.scalar)):
                            lo = pi * half
                            nc.vector.tensor_scalar(
                                out=val_b[:sw, lo:lo + half],
                                in0=ps_b[:sw, lo:lo + half],
                                scalar1=-VAL_OFF, scalar2=0.0,
                                op0=mybir.AluOpType.add, op1=mybir.AluOpType.max,
                            )
                            eng.dma_start(
                                vals[c0 + si:c0 + si + sw, QW + lo:QW + lo + half],
                                val_b[:sw, lo:lo + half],
                            )
                c0 += cw

    # Upgrade the placeholder waits now that the Tile scheduler has run.
    for w, sem in gate_waits:
        w.wait_op(sem, 16, "sem-ge")

    nc.compile()
    return nc


def _build_program_fp8_v4(n_shard: int = N_SHARD):
    """fp8e4m3 DoubleRow matmul, v4: resident bank + query-pass split.

    Inputs (packed on host; d-row r -> partition r % 128, plane r // 128):
      featT8  [128, 2, 8, 512]: featT8[p, qh, j, c] = feature[512*qh + c, 128*j + p]
      bank8cm [128, 8 * n_shard]: chunk-major per CHUNKS_V3 (see v3 docstring).
    Output: vals [n_shard, B] u8 = clamp(round(sim - VAL_OFF), 0, 255).

    Structure: the whole bank shard is DMA'd once into a resident SBUF tensor
    [128, 8, N_RES] (~100KB/partition) and the matmul stream makes two passes
    over it -- all subtiles for query half 0, then all for half 1.  Each pass
    writes vals[:, qh*512:(qh+1)*512].  The stream's gate is only c0 (128
    cols) + the two fq0 plane-pair pieces (split across both rings), so the
    first real matmul runs at ~11.4us; fq1 and the remaining chunks stream in
    far ahead of their consumption.  Warmup MMs bridge ~6.3us -> gate land
    with no PE-idle gap, keeping the HAM clock at 8/8 for the whole stream.
    Init memsets + barrier are stripped (_strip_init_overhead).  The final
    subtile of pass 2 drains as two 256-col DVE+store pieces on both rings.
    """
    assert n_shard == sum(CHUNKS_V5)
    nc = bacc.Bacc("TRN2", target_bir_lowering=False, debug=False)
    _strip_init_overhead(nc)

    featT8 = nc.dram_tensor(
        "featT8", [P, 2, KCH, QW], mybir.dt.float8e4, kind="ExternalInput"
    )
    n_padded = sum(((cw + 15) // 16) * 16 for cw in CHUNKS_V5)
    bank8cm = nc.dram_tensor(
        "bank8cm", [P, KCH * n_padded], mybir.dt.float8e4, kind="ExternalInput"
    )
    vals = nc.dram_tensor("vals", [n_shard, B], mybir.dt.uint8, kind="ExternalOutput")

    DR = mybir.MatmulPerfMode.DoubleRow
    NCH = len(CHUNKS_V5)

    wsrc = nc.alloc_sbuf_tensor("warm_src", [P, 2, QW], mybir.dt.float8e4)
    wsrc_ap = wsrc[:, :, :]

    # Resident operands.  One SBUF tensor PER CHUNK so every chunk DMA writes
    # a contiguous per-partition range: a strided dst (slice of one big
    # resident tensor) needs ~1024 descriptors and its HWDGE desc-gen takes
    # 5-10us per chunk, serializing the whole load queue (v4's 27us stall).
    # The 84-col chunk is padded to 96 (DoubleRow needs plane step % 16 == 0);
    # the DRAM side is padded identically by the host packer.
    cwp = [((cw + 15) // 16) * 16 for cw in CHUNKS_V5]
    bank_c = [
        nc.alloc_sbuf_tensor(f"bank_c{i}", [P, KCH, cwp[i]], mybir.dt.float8e4)
        for i in range(NCH)
    ]
    fq0_r = nc.alloc_sbuf_tensor("fq0_r", [P, KCH, QW], mybir.dt.float8e4)
    fq1_r = nc.alloc_sbuf_tensor("fq1_r", [P, KCH, QW], mybir.dt.float8e4)

    s_c = [nc.alloc_semaphore(f"s_c{i}") for i in range(NCH)]
    s_f0 = [nc.alloc_semaphore(f"s_f0_{k}") for k in range(KK)]
    s_f1 = [nc.alloc_semaphore(f"s_f1_{k}") for k in range(KK)]

    poffs = []  # padded offsets into bank8cm
    o = 0
    for w in cwp:
        poffs.append(o)
        o += w

    def chunk_dma(eng, ci):
        eng.dma_start(
            bank_c[ci][:, :, :],
            bank8cm[:, KCH * poffs[ci]:KCH * (poffs[ci] + cwp[ci])],
        ).then_inc(s_c[ci], 16)

    def feat_dma(eng, qh, kk):
        fq = (fq0_r, fq1_r)[qh]
        sem = (s_f0, s_f1)[qh][kk]
        eng.dma_start(
            fq[:, 2 * kk:2 * kk + 2, :], featT8[:, qh, 2 * kk:2 * kk + 2, :]
        ).then_inc(sem, 16)

    # Gate loads, earliest-need-first per ring; featT streams per-plane-pair
    # (128K pieces) alternating rings so the first matmul gates on only
    # c0 + the first fq0 quarter (~256K).
    # scalar: c0, fq0 even quarters, c2, c4, c5, fq1 even, bulk chunks.
    # sync:   c1, fq0 odd quarters, c3, fq1 odd, then all stores (in-context).
    chunk_dma(nc.scalar, 0)
    chunk_dma(nc.sync, 1)
    feat_dma(nc.scalar, 0, 0)
    feat_dma(nc.sync, 0, 1)
    feat_dma(nc.scalar, 0, 2)
    feat_dma(nc.sync, 0, 3)
    chunk_dma(nc.scalar, 2)
    chunk_dma(nc.sync, 3)
    chunk_dma(nc.scalar, 4)
    chunk_dma(nc.scalar, 5)
    feat_dma(nc.sync, 1, 1)
    feat_dma(nc.sync, 1, 3)
    feat_dma(nc.scalar, 1, 0)
    feat_dma(nc.scalar, 1, 2)
    for ci in range(N_RAMP_V5 + 1, NCH):
        chunk_dma(nc.scalar, ci)

    gate_waits = []  # (BassInstruction placeholder, sem) -> upgraded to >= 16

    with nc.psum_tensor([P, QW], mybir.dt.float32) as wps:
        wps_ap = wps[:, :]
        for _ in range(N_WARM_V5[0]):
            nc.tensor.matmul(
                wps_ap, wsrc_ap[:, :, 0:P], wsrc_ap,
                start=True, stop=True, perf_mode=DR,
            )
        for _ in range(N_WARM_V5[1]):
            nc.tensor.matmul(
                wps_ap[:, 0:P], wsrc_ap[:, :, 0:P], wsrc_ap[:, :, 0:P],
                start=True, stop=True, perf_mode=DR,
            )

    featq = [fq0_r, fq1_r]

    # Pass schedule: q0-only ramp over chunks 0..4 while fq1 is in flight,
    # dual-pass per chunk for 5..10, then the deferred q1 sweeps of 0..4
    # (DMA-free) and chunk 11 (84 cols) as the final small piece.
    schedule = [(ci, 0) for ci in range(N_RAMP_V5)]
    for ci in range(N_RAMP_V5, NCH - 1):
        schedule += [(ci, 0), (ci, 1)]
    schedule += [(ci, 1) for ci in range(N_RAMP_V5)]
    schedule += [(NCH - 1, 0), (NCH - 1, 1)]
    first_q1 = next(i for i, (ci, qh) in enumerate(schedule) if qh == 1)

    offs = []  # unpadded global column offsets (vals addressing)
    o = 0
    for cw in CHUNKS_V5:
        offs.append(o)
        o += cw

    with tile.TileContext(nc) as tc:
        with (
            tc.tile_pool(name="valp", bufs=6) as val_pool,
            tc.tile_pool(name="psum", bufs=6, space=bass.MemorySpace.PSUM) as psum_pool,
        ):
            seen_chunk = set()
            for ent_i, (ci, qh) in enumerate(schedule):
                cw = CHUNKS_V5[ci]
                qs = slice(qh * QW, (qh + 1) * QW)
                last_ent = ent_i == len(schedule) - 1
                for si in range(0, cw, P):
                    sw = min(P, cw - si)
                    gco = offs[ci] + si
                    last = last_ent and si + P >= cw

                    ps = psum_pool.tile([P, QW], mybir.dt.float32)
                    val_t = val_pool.tile([P, QW], mybir.dt.uint8)
                    for kk in range(KK):
                        mm = nc.tensor.matmul(
                            ps[:sw, :],
                            bank_c[ci][:, 2 * kk:2 * kk + 2, si:si + sw],
                            featq[qh][:, 2 * kk:2 * kk + 2, :],
                            start=(kk == 0),
                            stop=(kk == KK - 1),
                            perf_mode=DR,
                        )
                        # Gate waits are attached DIRECTLY to the first
                        # consuming matmuls (upgraded to >=16 post-scheduling)
                        # -- free-floating placeholder EVENT_SEMAPHOREs have
                        # no data deps, so the Tile scheduler bunches them
                        # arbitrarily, which serialized the whole stream.
                        if si == 0 and kk == 0 and ci not in seen_chunk:
                            seen_chunk.add(ci)
                            gate_waits.append((mm, s_c[ci]))
                        if ent_i == 0 and si == 0:
                            gate_waits.append((mm, s_f0[kk]))
                        if ent_i == first_q1 and si == 0:
                            gate_waits.append((mm, s_f1[kk]))
                    if not last:
                        nc.vector.tensor_scalar(
                            out=val_t[:sw, :],
                            in0=ps[:sw, :],
                            scalar1=-VAL_OFF,
                            scalar2=0.0,
                            op0=mybir.AluOpType.add,
                            op1=mybir.AluOpType.max,
                        )
                        nc.sync.dma_start(
                            vals[gco:gco + sw, qs], val_t[:sw, :]
                        )
                    else:
                        # Final piece: two 256-col DVE+store pieces on both
                        # rings shorten the post-stream drain.
                        half = QW // 2
                        for pi, eng in ((0, nc.sync), (1, nc.scalar)):
                            lo = pi * half
                            nc.vector.tensor_scalar(
                                out=val_t[:sw, lo:lo + half],
                                in0=ps[:sw, lo:lo + half],
                                scalar1=-VAL_OFF, scalar2=0.0,
                                op0=mybir.AluOpType.add,
                                op1=mybir.AluOpType.max,
                            )
                            eng.dma_start(
                                vals[gco:gco + sw, qh * QW + lo:
                                     qh * QW + lo + half],
                                val_t[:sw, lo:lo + half],
                            )

    for w, sem in gate_waits:
        # check=False: the MM may already carry a scheduler wait; compile()'s
        # generate_event_semaphores pass legalizes multi-wait instructions by
        # splitting into an adjacent EVENT_SEMAPHORE.
        w.wait_op(sem, 16, "sem-ge", check=False)

    nc.compile()
    return nc


def _build_program_fp8_v0(n_shard: int = N_SHARD):
    """Previous fp8 layout (unpacked inputs, 8 DMAs per chunk). Fallback."""
    nc = bacc.Bacc("TRN2", target_bir_lowering=False, debug=False)

    featT8 = nc.dram_tensor("featT8", [D, B], mybir.dt.float8e4, kind="ExternalInput")
    bank8 = nc.dram_tensor("bank8", [D, n_shard], mybir.dt.float8e4, kind="ExternalInput")
    vals = nc.dram_tensor("vals", [n_shard, B], mybir.dt.uint8, kind="ExternalOutput")

    CH8 = 1024  # bank cols per DMA chunk (1 KiB fp8 rows)
    with tile.TileContext(nc) as tc:
        with (
            tc.tile_pool(name="feat", bufs=1) as feat_pool,
            tc.tile_pool(name="bankp", bufs=4) as bank_pool,
            tc.tile_pool(name="valp", bufs=8) as val_pool,
            tc.tile_pool(name="psum", bufs=4, space=bass.MemorySpace.PSUM) as psum_pool,
        ):
            featT_sb = feat_pool.tile([P, KK, 2, B], mybir.dt.float8e4)
            for kk in range(KK):
                for i in range(2):
                    r0 = (2 * kk + i) * P
                    nc.sync.dma_start(featT_sb[:, kk, i, :], featT8[r0:r0 + P, :])

            nch = (n_shard + CH8 - 1) // CH8
            for ci in range(nch):
                c0 = ci * CH8
                cw = min(CH8, n_shard - c0)
                bank_sb = bank_pool.tile([P, KK, 2, CH8], mybir.dt.float8e4)
                for kk in range(KK):
                    for i in range(2):
                        r0 = (2 * kk + i) * P
                        nc.sync.dma_start(
                            bank_sb[:, kk, i, :cw], bank8[r0:r0 + P, c0:c0 + cw]
                        )
                for si in range(0, cw, P):
                    sw = min(P, cw - si)
                    val_t = val_pool.tile([P, B], mybir.dt.uint8)
                    ps = psum_pool.tile([P, B], mybir.dt.float32)  # 2 PSUM banks
                    for qh in range(B // QW):
                        for kk in range(KK):
                            nc.tensor.matmul(
                                ps[:sw, qh * QW:(qh + 1) * QW],
                                bank_sb[:, kk, :, si:si + sw],
                                featT_sb[:, kk, :, qh * QW:(qh + 1) * QW],
                                start=(kk == 0),
                                stop=(kk == KK - 1),
                                perf_mode=mybir.MatmulPerfMode.DoubleRow,
                            )
                    nc.vector.tensor_scalar(
                        out=val_t[:sw, :],
                        in0=ps[:sw, :],
                        scalar1=-VAL_OFF,
                        scalar2=0.0,
                        op0=mybir.AluOpType.add,
                        op1=mybir.AluOpType.max,
                    )
                    nc.sync.dma_start(vals[c0 + si:c0 + si + sw, :], val_t[:sw, :])

    nc.compile()
    return nc


_PROGRAM_CACHE = {}


def _get_program(impl, n_shard):
    key = (impl, n_shard)
    if key not in _PROGRAM_CACHE:
        build = {
            "fp8": _build_program_fp8_v4,
            "fp8v3": _build_program_fp8_v3,
            "fp8v2": _build_program_fp8,
            "fp8v0": _build_program_fp8_v0,
            "bf16": _build_program_bf16,
        }[impl]
        _PROGRAM_CACHE[key] = build(n_shard)
    return _PROGRAM_CACHE[key]


def _profile_ctx():
    import contextlib

    @contextlib.contextmanager
    def _maybe_profile():
        """Optional NTFF capture via the axon NRT-profile C ABI."""
        prof_dir = os.environ.get("KNN_PROFILE_DIR")
        if not prof_dir:
            yield
            return
        import ctypes
        lib = ctypes.CDLL("/opt/axon/libaxon_pjrt.so")
        lib.axon_start_nrt_profile.argtypes = [
            ctypes.POINTER(ctypes.c_int64), ctypes.c_size_t]
        lib.axon_start_nrt_profile.restype = ctypes.c_int64
        lib.axon_stop_nrt_profile.argtypes = [ctypes.c_char_p]
        lib.axon_stop_nrt_profile.restype = ctypes.c_int64
        import jax
        jax.devices()
        rc = lib.axon_start_nrt_profile(None, 0)
        if rc != 0:
            raise RuntimeError(f"axon_start_nrt_profile rc={rc}")
        try:
            yield
        finally:
            n = lib.axon_stop_nrt_profile(str(prof_dir).encode())
            print(f"ntff profile: {n} file(s) -> {prof_dir}", flush=True)

    return _maybe_profile()


def _run_spmd(nc, in_maps):
    global LAST_EXEC_TIME_NS
    with _profile_ctx():
        res = run_bass_kernel_spmd(
            nc, in_maps, core_ids=list(range(N_CORES)), trace=False
        )
    LAST_EXEC_TIME_NS = res.exec_time_ns
    _tlog("device run done")
    return res


def _candidate_pairs_bf16(feature, bank_f32):
    """bf16+mask path: device mask -> all candidate pairs."""
    n = bank_f32.shape[1]
    n_shard = n // N_CORES
    nc = _get_program("bf16", n_shard)
    _tlog("program built")

    featT_bf = np.ascontiguousarray(feature.T).astype(ml_dtypes.bfloat16)
    bank_bf = bank_f32.astype(ml_dtypes.bfloat16)
    in_maps = [
        {
            "featT": featT_bf,
            "bank": np.ascontiguousarray(bank_bf[:, i * n_shard:(i + 1) * n_shard]),
        }
        for i in range(N_CORES)
    ]
    res = _run_spmd(nc, in_maps)
    mask = np.concatenate([res.results[i]["mask"] for i in range(N_CORES)], axis=0)

    nidx, qidx = np.nonzero(mask)  # [N, B]: sorted by bank idx
    order = np.argsort(qidx, kind="stable")  # per-query segments, nidx ascending
    qidx = qidx[order]
    nidx = nidx[order]
    counts = np.bincount(qidx, minlength=feature.shape[0])
    starts = np.zeros(feature.shape[0] + 1, dtype=np.int64)
    np.cumsum(counts, out=starts[1:])
    _tlog(f"candidates built ({len(nidx)} pairs)")
    return qidx, nidx, starts


def _candidate_pairs_fp8(feature, bank_f32, k, impl="fp8"):
    """fp8+values path: threshold, then keep only the top-k confidence window."""
    global LAST_DEV_VALS
    n = bank_f32.shape[1]
    n_shard = n // N_CORES
    nc = _get_program(impl, n_shard)
    _tlog("program built")

    f8 = feature.astype(ml_dtypes.float8_e4m3)
    bank_8 = bank_f32.astype(ml_dtypes.float8_e4m3)
    if impl in ("fp8", "fp8v3"):
        # v3/v5 packed layouts (see _build_program_fp8_v3/v4 docstrings).
        chunks = CHUNKS_V5 if impl == "fp8" else CHUNKS_V3
        featT_pack = np.ascontiguousarray(
            f8.T.reshape(KCH, P, 2, QW).transpose(1, 2, 0, 3)
        )

        def _pack_bank_cm(shard):
            # chunk-major: each chunk's 8 planes contiguous per partition.
            # v5 ("fp8") pads each chunk width to a multiple of 16 (DR stride
            # rule); pad columns are zero and never read by the matmuls.
            pad16 = impl == "fp8"
            widths = [(((cw + 15) // 16) * 16 if pad16 else cw) for cw in chunks]
            out = np.zeros((P, KCH * sum(widths)), dtype=shard.dtype)
            o = 0
            po = 0
            for cw, pw in zip(chunks, widths):
                out[:, KCH * po:KCH * po + KCH * pw].reshape(P, KCH, pw)[:, :, :cw] = (
                    shard[:, o:o + cw].reshape(KCH, P, cw).transpose(1, 0, 2)
                )
                o += cw
                po += pw
            assert o == n_shard
            return out

        in_maps = [
            {
                "featT8": featT_pack,
                "bank8cm": _pack_bank_cm(bank_8[:, i * n_shard:(i + 1) * n_shard]),
            }
            for i in range(N_CORES)
        ]
    elif impl == "fp8v2":
        # Packed layouts (see _build_program_fp8 docstring).
        featT_pack = np.ascontiguousarray(
            f8.T.reshape(KCH, P, 2, QW).transpose(1, 2, 0, 3)
        )

        def _pack_bank(shard):
            # d-row r -> partition r % 128, plane r // 128.
            return np.ascontiguousarray(
                shard.reshape(KCH, P, n_shard).transpose(1, 0, 2)
            )

        def _pack_bank_head(shard):
            # chunks 0+1 duplicated contiguous-per-partition (big DMA lines).
            cw0, cw1 = CHUNKS_V2[0], CHUNKS_V2[1]
            out = np.empty((P, KCH * (cw0 + cw1)), dtype=shard.dtype)
            out[:, :KCH * cw0] = (
                shard[:, :cw0].reshape(KCH, P, cw0)
                .transpose(1, 0, 2).reshape(P, KCH * cw0)
            )
            out[:, KCH * cw0:] = (
                shard[:, cw0:cw0 + cw1].reshape(KCH, P, cw1)
                .transpose(1, 0, 2).reshape(P, KCH * cw1)
            )
            return out

        in_maps = [
            {
                "featT8": featT_pack,
                "bank8": _pack_bank(bank_8[:, i * n_shard:(i + 1) * n_shard]),
                "bank8head": _pack_bank_head(
                    bank_8[:, i * n_shard:(i + 1) * n_shard]
                ),
            }
            for i in range(N_CORES)
        ]
    else:
        featT_8 = np.ascontiguousarray(f8.T)
        in_maps = [
            {
                "featT8": featT_8,
                "bank8": np.ascontiguousarray(bank_8[:, i * n_shard:(i + 1) * n_shard]),
            }
            for i in range(N_CORES)
        ]
    _tlog("inputs packed")
    res = _run_spmd(nc, in_maps)
    vals = np.concatenate([res.results[i]["vals"] for i in range(N_CORES)], axis=0)
    LAST_DEV_VALS = vals  # [N, B] u8: clamp(sim - VAL_OFF, 0, 255)

    m = vals >= np.uint8(T0_FP8_U8)
    nidx, qidx = np.nonzero(m)
    # The DVE f32->u8 cast rounds-to-nearest, so stored+VAL_OFF is already the
    # quantization-interval midpoint (E_FP8 covers the +-0.5 either way).
    dv = vals[nidx, qidx].astype(np.float32) + np.float32(VAL_OFF)
    order = np.argsort(qidx, kind="stable")  # per-query segments, nidx ascending
    qidx = qidx[order]
    nidx = nidx[order]
    dv = dv[order]
    b = feature.shape[0]
    counts = np.bincount(qidx, minlength=b)
    starts_all = np.zeros(b + 1, dtype=np.int64)
    np.cumsum(counts, out=starts_all[1:])
    _tlog(f"thresholded ({len(nidx)} pairs)")

    # Per query, keep only candidates that can possibly be in the true top-k:
    # dev >= dev_rank_k - 2E (see module docstring for the bound).
    keep = np.zeros(len(nidx), dtype=bool)
    for q in range(b):
        s, e = starts_all[q], starts_all[q + 1]
        c = e - s
        if c < k:
            keep[s:e] = True  # top-k loop will take the full-row fallback
            continue
        seg = dv[s:e]
        rk = np.partition(seg, c - k)[c - k]
        keep[s:e] = seg >= rk - 2.0 * E_FP8
    qidx = qidx[keep]
    nidx = nidx[keep]
    counts = np.bincount(qidx, minlength=b)
    starts = np.zeros(b + 1, dtype=np.int64)
    np.cumsum(counts, out=starts[1:])
    _tlog(f"windowed ({len(nidx)} pairs)")
    return qidx, nidx, starts


def _finish(feature, bank_f32, labels, num_classes, k, cand):
    """Exact fp32 re-rank of candidate pairs + reference post-processing.

    cand is (qidx, nidx, starts) or None (full host fallback).
    """
    b, d = feature.shape
    n = bank_f32.shape[1]

    if cand is not None:
        qidx, nidx, starts = cand
        bankT = np.ascontiguousarray(bank_f32.T)  # contiguous row gathers
        _tlog("bankT transpose done")
        vals = np.empty(len(nidx), dtype=np.float32)
        CHP = 1 << 16
        for s in range(0, len(nidx), CHP):
            e = min(s + CHP, len(nidx))
            vals[s:e] = np.einsum(
                "ij,ij->i", feature[qidx[s:e]], bankT[nidx[s:e]]
            )
        _tlog(f"exact vals done ({len(nidx)} pairs)")

    full_rows = None
    full_q0 = 0
    all_idx = np.arange(n)

    sel_q = np.empty(b * k, dtype=np.int64)
    sel_lab = np.empty(b * k, dtype=np.int64)
    sel_val = np.empty(b * k, dtype=np.float32)
    pos = 0
    ROWBLK = 64
    for q in range(b):
        if cand is not None and starts[q + 1] - starts[q] >= k:
            s, e = starts[q], starts[q + 1]
            v = vals[s:e]
            idx = nidx[s:e]
        else:
            # Exact full row (no device pre-filter, or threshold miss).
            if full_rows is None or not (full_q0 <= q < full_q0 + ROWBLK):
                full_q0 = q
                hi = min(q + ROWBLK, b)
                full_rows = feature[q:hi] @ bank_f32
            v = full_rows[q - full_q0]
            idx = all_idx
        # jax.lax.top_k semantics: descending, ties -> lower index first.
        order = np.argsort(-v, kind="stable")[:k]
        sel_q[pos:pos + k] = q
        sel_lab[pos:pos + k] = labels[idx[order]]
        sel_val[pos:pos + k] = v[order]
        pos += k
    _tlog("per-query topk done")

    with np.errstate(over="ignore"):
        w = np.exp(sel_val / np.float32(KNN_T)).astype(np.float32)
    scores = np.zeros((b, num_classes), dtype=np.float32)
    np.add.at(scores, (sel_q, sel_lab), w)
    _tlog("scatter done")
    return scores


def kernel(feature, feature_bank, feature_labels, num_classes, knn_k):
    _tlog("kernel() start")
    feature = np.asarray(feature, dtype=np.float32)
    bank_f32 = np.asarray(feature_bank, dtype=np.float32)
    labels = np.asarray(feature_labels)
    c = int(np.asarray(num_classes))
    k = int(np.asarray(knn_k))

    b, d = feature.shape
    n = bank_f32.shape[1]

    impl = os.environ.get("KNN_IMPL", "fp8")
    use_device = d == D and b == B and n % N_CORES == 0 and n // N_CORES > 0
    if use_device and impl in ("fp8", "fp8v3") and n // N_CORES != sum(CHUNKS_V3):
        impl = "fp8v0"  # packed layout is hardcoded for the spec shard size
    if use_device and impl == "fp8v2" and n // N_CORES != sum(CHUNKS_V2):
        impl = "fp8v0"
    if use_device:
        if impl in ("fp8", "fp8v3", "fp8v2", "fp8v0"):
            cand = _candidate_pairs_fp8(feature, bank_f32, k, impl)
        else:
            cand = _candidate_pairs_bf16(feature, bank_f32)
    else:
        cand = None  # degenerate fallback: host does it all

    scores = _finish(feature, bank_f32, labels, c, k, cand)
    pred = np.argsort(-scores, axis=1, kind="stable").astype(np.int32)
    _tlog("final argsort done")
    return pred

